# revision 23
# baseline (speedup 1.0000x reference)
"""Bahdanau attention kernel for Trainium2, SPMD over 8 NeuronCores.

Problem shapes: features [32, 2048, 1024] f32, hidden [32, 1024] f32,
W1/W2 [1024, 1024], b1/b2 [1024], V [1024, 1], bv [1].

Returns (context_vector [32, 1024] f32, attention_weights [32, 2048, 1] f32).

Sharding: data-parallel over batch B; each of the 8 cores handles 4 batches
end-to-end (no collectives needed).

Per-core pipeline, per batch b (T=2048 split into 4 chunks of 512 rows):
  1. SWDGE cast-DMA loads the F chunk f32->bf16 as FB [128(tp), 4(o), 1024(d)].
  2. DMA-xbar transposes the whole chunk: ft[p, o, j, c] = F^T[j*128+p, o*128+c].
  3. projT[u,t] = sum_j W1[dj,u].T @ FT[dj,t]  (bf16 matmul, PSUM f32).
  4. ScalarE tanh(projT + bh[u]) -> scoreT bf16, where bh = hidden@W2 + b1 + b2
     enters as the per-partition activation bias (free).
  5. logits[1,t] = sum_m V[um].T @ scoreT[um,t]  (matmul, M=1).
  6. Online softmax without max-subtraction (tanh bounds |logits| <= sum|V|
     ~ 25, so exp is safe in f32): per chunk, logits are PE-transposed to
     t-partition layout, exponentiated, and immediately contracted with FB
     into two running context PSUM accumulators [1, 512].  The chunk's
     transposes/exp/context matmuls are emitted one chunk later so the PE
     never waits on the ScalarE chain.
  7. Batch end: exp over the full logit row (accum_out gives the softmax
     denominator), reciprocal, normalize weights in place, scale the context
     accumulators.
"""

import ml_dtypes
import numpy as np

import concourse.bass as bass
import concourse.mybir as mybir
import concourse.tile as tile
from concourse import bacc
from concourse.bass_utils import run_bass_kernel_spmd

N_CORES = 8
B_LOC = 4  # batches per core
T = 2048
D = 1024
U = 1024
CHUNK = 512  # t rows per chunk
N_CHUNKS = T // CHUNK  # 4
O_PER_CHUNK = CHUNK // 128  # 4 t-subtiles per chunk
NJ = D // 128  # 8 d-tiles
NM = U // 128  # 8 u-tiles

F32 = mybir.dt.float32
BF16 = mybir.dt.bfloat16
AX = mybir.AxisListType
AF = mybir.ActivationFunctionType


def build_kernel():
    nc = bacc.Bacc("TRN2", target_bir_lowering=False, debug=False,
                   num_devices=N_CORES)

    feats = nc.dram_tensor("features", [B_LOC, T, D], BF16, kind="ExternalInput")
    hidden = nc.dram_tensor("hidden", [B_LOC, D], BF16, kind="ExternalInput")
    w1 = nc.dram_tensor("W1", [D, U], BF16, kind="ExternalInput")
    w2 = nc.dram_tensor("W2", [D, U], BF16, kind="ExternalInput")
    b1 = nc.dram_tensor("b1", [U], F32, kind="ExternalInput")
    b2 = nc.dram_tensor("b2", [U], F32, kind="ExternalInput")
    v = nc.dram_tensor("V", [U, 1], F32, kind="ExternalInput")

    ctx_out = nc.dram_tensor("ctx", [B_LOC, D], F32, kind="ExternalOutput")
    w_out = nc.dram_tensor("w", [B_LOC, T], F32, kind="ExternalOutput")

    with tile.TileContext(nc) as tc:
        with (
            tc.tile_pool(name="const", bufs=1) as cpool,
            tc.tile_pool(name="fb", bufs=8) as fb_pool,
            tc.tile_pool(name="ft", bufs=5) as ft_pool,
            tc.tile_pool(name="score", bufs=3) as sc_pool,
            tc.tile_pool(name="small", bufs=2) as sm_pool,
            tc.tile_pool(name="ps_proj", bufs=3, space="PSUM") as ps_proj,
            tc.tile_pool(name="ps_lg", bufs=2, space="PSUM") as ps_lg,
            tc.tile_pool(name="ps_pst", bufs=1, space="PSUM") as ps_pst,
            tc.tile_pool(name="ps_ctx", bufs=2, space="PSUM") as ps_ctx,
        ):
            # ---- constants / weights in SBUF --------------------------------
            ident1 = cpool.tile([1, 1], F32, tag="ident1")
            nc.vector.memset(ident1[:], 1.0)

            # the first two chunk transposes go first: the xbar-mode rule
            # serializes a transpose behind all in-flight DMA copies, so any
            # copy emitted before them would delay the first matmuls.
            pre_ft = {}
            for c0 in range(1):
                ftp = ft_pool.tile([128, NJ, CHUNK], BF16, tag="ft",
                                   name=f"ft_pre{c0}")
                nc.sync.dma_start_transpose(
                    ftp[:], feats[0, c0 * CHUNK:(c0 + 1) * CHUNK, :])
                pre_ft[(0, c0)] = ftp

            # small natural (contiguous) loads: they are cheap and feed
            # the bh build; scatter layouts are produced on-chip instead.
            hTn = cpool.tile([128, B_LOC, NJ], BF16, tag="ht")  # [dp, b, j]
            nc.gpsimd.dma_start(hTn[:], hidden.rearrange("b (j p) -> p b j", p=128))
            b1row = cpool.tile([1, U], F32, tag="b1row")
            nc.sync.dma_start(b1row[:], b1.rearrange("(o u) -> o u", o=1))
            b2row = cpool.tile([1, U], F32, tag="b2row")
            nc.sync.dma_start(b2row[:], b2.rearrange("(o u) -> o u", o=1))
            vrow = cpool.tile([1, U], F32, tag="vrow")
            nc.sync.dma_start(vrow[:], v.rearrange("u o -> o u"))

            w1sb = cpool.tile([128, NJ, U], BF16, tag="w1")  # [dp, j, u]
            for qf in range(4):
                nc.scalar.dma_start(
                    w1sb[:, qf * 2:(qf + 1) * 2, :],
                    w1[qf * 256:(qf + 1) * 256, :]
                    .rearrange("(j p) u -> p j u", p=128))

            # W2 slabs early so the bh matmuls don't stall mid-stream
            w2slabs = []
            for half in range(2):
                w2slab = ft_pool.tile([128, NJ, U // 2], BF16, tag="ft",
                                      name=f"w2slab{half}")
                nc.gpsimd.dma_start(
                    w2slab[:],
                    w2[:, half * 512:(half + 1) * 512]
                    .rearrange("(j p) u -> p j u", p=128),
                )
                w2slabs.append(w2slab)

            bh = cpool.tile([128, NM, B_LOC], F32, tag="bh")
            vsb = cpool.tile([128, NM], BF16, tag="v")  # [up, m]

            def build_bh():
                """bh[u, m, b] = (hidden @ W2)^T + b1 + b2; V to [up, m]."""
                # b1+b2 and V rows -> partition layout via K=1 PE transposes
                b12row = cpool.tile([1, U], F32, tag="b12row")
                nc.vector.tensor_add(b12row[:], b1row[:], b2row[:])
                psi = ps_proj.tile([128, CHUNK], F32, tag="proj", name="psinit")
                for m in range(NM):
                    nc.tensor.transpose(psi[:, m:m + 1],
                                        b12row[0:1, m * 128:(m + 1) * 128], ident1)
                for m in range(NM):
                    nc.tensor.transpose(psi[:, NM + m:NM + m + 1],
                                        vrow[0:1, m * 128:(m + 1) * 128], ident1)
                b12T = cpool.tile([128, NM], F32, tag="b12")
                nc.scalar.copy(b12T[:], psi[:, :NM])
                nc.scalar.copy(vsb[:], psi[:, NM:2 * NM])

                for half in range(2):
                    w2slab = w2slabs[half]
                    for mm in range(NM // 2):
                        m = half * 4 + mm
                        ps = ps_proj.tile([128, CHUNK], F32, tag="proj")
                        for j in range(NJ):
                            nc.tensor.matmul(
                                ps[:, :B_LOC],
                                lhsT=w2slab[:, j, mm * 128:(mm + 1) * 128],
                                rhs=hTn[:, :, j],
                                start=(j == 0),
                                stop=(j == NJ - 1),
                            )
                        nc.scalar.activation(bh[:, m, :], ps[:, :B_LOC],
                                             AF.Identity, bias=b12T[:, m:m + 1])

            # ---- main loop --------------------------------------------------
            def main_phase_chunk(b, c, la, ctx_ps, mid_hook=None):
                """Load + transpose + proj + tanh + logits for chunk c; defer
                the softmax tail + context matmuls to the next chunk."""
                # FT straight from DRAM through the xbar: ft[p, j, t] = F^T[j*128+p, t]
                ft = pre_ft.pop((b, c), None)
                if ft is None:
                    ft = ft_pool.tile([128, NJ, CHUNK], BF16, tag="ft")
                    nc.sync.dma_start_transpose(
                        ft[:], feats[b, c * CHUNK:(c + 1) * CHUNK, :])

                # natural-layout copy for the context matmuls (1-chunk slack)
                fb = fb_pool.tile([128, O_PER_CHUNK, D], BF16, tag="fb")
                nc.gpsimd.dma_start(
                    fb[:],
                    feats[b, c * CHUNK:(c + 1) * CHUNK, :]
                    .rearrange("(o p) d -> p o d", p=128),
                )

                # projT (per u-tile) -> tanh -> scoreT
                score = sc_pool.tile([128, NM, CHUNK], BF16, tag="score")
                for m in range(NM):
                    ps = ps_proj.tile([128, CHUNK], F32, tag="proj")
                    for j in range(NJ):
                        nc.tensor.matmul(
                            ps[:],
                            lhsT=w1sb[:, j, m * 128:(m + 1) * 128],
                            rhs=ft[:, j, :],
                            start=(j == 0),
                            stop=(j == NJ - 1),
                        )
                    if m == 0 and mid_hook is not None:
                        mid_hook()
                    nc.scalar.activation(score[:, m, :], ps[:], AF.Tanh,
                                         bias=bh[:, m, b:b + 1])

                psl = ps_lg.tile([1, CHUNK], F32, tag="lg")
                for m in range(NM):
                    nc.tensor.matmul(
                        psl[:],
                        lhsT=vsb[:, m:m + 1],
                        rhs=score[:, m, :],
                        start=(m == 0),
                        stop=(m == NM - 1),
                    )
                nc.scalar.copy(la[:, c * CHUNK:(c + 1) * CHUNK], psl[:])

                def post():
                    # logits chunk -> t-partition layout -> exp -> context MMs
                    pst = ps_pst.tile([128, O_PER_CHUNK], F32, tag="pst")
                    for o in range(O_PER_CHUNK):
                        blk = la[0:1, c * CHUNK + o * 128: c * CHUNK + (o + 1) * 128]
                        nc.tensor.transpose(pst[:, o:o + 1], blk, ident1)
                    ewc = sm_pool.tile([128, O_PER_CHUNK], BF16, tag="ewc")
                    nc.scalar.activation(ewc[:], pst[:], AF.Exp)
                    for h in range(2):
                        for o in range(O_PER_CHUNK):
                            nc.tensor.matmul(
                                ctx_ps[h][:],
                                lhsT=ewc[:, o:o + 1],
                                rhs=fb[:, o, h * 512:(h + 1) * 512],
                                start=(c == 0 and o == 0),
                                stop=(c == N_CHUNKS - 1 and o == O_PER_CHUNK - 1),
                            )

                return post

            def batch_finish(b, la, ctx_ps):
                """exp + denominator, normalized weights out, scaled context out."""
                ew = sm_pool.tile([1, T], F32, tag="ew")
                ssum = sm_pool.tile([1, 1], F32, tag="ssum")
                nc.scalar.activation(ew[:], la[:], AF.Exp, accum_out=ssum[:])
                s_inv = sm_pool.tile([1, 1], F32, tag="sinv")
                nc.vector.reciprocal(s_inv[:], ssum[:])
                nc.vector.tensor_scalar_mul(ew[:], ew[:], s_inv[:, 0:1])
                nc.gpsimd.dma_start(w_out[b:b + 1, :], ew[:])

                ctx_sb = sm_pool.tile([1, D], F32, tag="ctx")
                for h in range(2):
                    nc.vector.tensor_scalar_mul(
                        ctx_sb[:, h * 512:(h + 1) * 512], ctx_ps[h][:],
                        s_inv[:, 0:1])
                nc.gpsimd.dma_start(ctx_out[b:b + 1, :], ctx_sb[:])

            pending = None  # previous chunk's deferred tail (one-chunk delay)
            for b in range(B_LOC):
                la = sm_pool.tile([1, T], F32, tag="la")
                ctx_ps = [ps_ctx.tile([1, CHUNK], F32, tag="ctx", name=f"ctxp{h}")
                          for h in range(2)]
                for c in range(N_CHUNKS):
                    hook = build_bh if (b == 0 and c == 0) else None
                    tail = main_phase_chunk(b, c, la, ctx_ps, mid_hook=hook)
                    if pending is not None:
                        pending()
                    pending = tail
                # batch end: flush last chunk's tail, then finish
                pending()
                pending = None
                batch_finish(b, la, ctx_ps)

    nc.compile()
    return nc


_NC_CACHE = None


def _get_nc():
    global _NC_CACHE
    if _NC_CACHE is None:
        _NC_CACHE = build_kernel()
    return _NC_CACHE


def kernel(**inputs):
    bf16 = ml_dtypes.bfloat16
    feats = np.ascontiguousarray(np.asarray(inputs["features"]).astype(bf16))
    hidden = np.ascontiguousarray(np.asarray(inputs["hidden"]).astype(bf16))
    w1 = np.ascontiguousarray(np.asarray(inputs["W1"]).astype(bf16))
    w2 = np.ascontiguousarray(np.asarray(inputs["W2"]).astype(bf16))
    b1 = np.ascontiguousarray(np.asarray(inputs["b1"], dtype=np.float32))
    b2 = np.ascontiguousarray(np.asarray(inputs["b2"], dtype=np.float32))
    v = np.ascontiguousarray(np.asarray(inputs["V"], dtype=np.float32))

    nc = _get_nc()
    in_maps = []
    for i in range(N_CORES):
        sl = slice(i * B_LOC, (i + 1) * B_LOC)
        in_maps.append({
            "features": feats[sl],
            "hidden": hidden[sl],
            "W1": w1,
            "W2": w2,
            "b1": b1,
            "b2": b2,
            "V": v,
        })
    res = run_bass_kernel_spmd(nc, in_maps, core_ids=list(range(N_CORES)))

    ctx = np.concatenate([res.results[i]["ctx"] for i in range(N_CORES)], axis=0)
    w = np.concatenate([res.results[i]["w"] for i in range(N_CORES)], axis=0)
    return ctx, w.reshape(N_CORES * B_LOC, T, 1)


# revision 26
# speedup vs baseline: 1.0130x; 1.0130x over previous
"""Bahdanau attention kernel for Trainium2, SPMD over 8 NeuronCores.

Problem shapes: features [32, 2048, 1024] f32, hidden [32, 1024] f32,
W1/W2 [1024, 1024], b1/b2 [1024], V [1024, 1], bv [1].

Returns (context_vector [32, 1024] f32, attention_weights [32, 2048, 1] f32).

Sharding: data-parallel over batch B; each of the 8 cores handles 4 batches
end-to-end (no collectives needed).

Per-core pipeline, per batch b (T=2048 split into 4 chunks of 512 rows):
  1. SWDGE cast-DMA loads the F chunk f32->bf16 as FB [128(tp), 4(o), 1024(d)].
  2. DMA-xbar transposes the whole chunk: ft[p, o, j, c] = F^T[j*128+p, o*128+c].
  3. projT[u,t] = sum_j W1[dj,u].T @ FT[dj,t]  (bf16 matmul, PSUM f32).
  4. ScalarE tanh(projT + bh[u]) -> scoreT bf16, where bh = hidden@W2 + b1 + b2
     enters as the per-partition activation bias (free).
  5. logits[1,t] = sum_m V[um].T @ scoreT[um,t]  (matmul, M=1).
  6. Online softmax without max-subtraction (tanh bounds |logits| <= sum|V|
     ~ 25, so exp is safe in f32): per chunk, logits are PE-transposed to
     t-partition layout, exponentiated, and immediately contracted with FB
     into two running context PSUM accumulators [1, 512].  The chunk's
     transposes/exp/context matmuls are emitted one chunk later so the PE
     never waits on the ScalarE chain.
  7. Batch end: exp over the full logit row (accum_out gives the softmax
     denominator), reciprocal, normalize weights in place, scale the context
     accumulators.
"""

import ml_dtypes
import numpy as np

import concourse.bass as bass
import concourse.mybir as mybir
import concourse.tile as tile
from concourse import bacc
from concourse.bass_utils import run_bass_kernel_spmd

N_CORES = 8
B_LOC = 4  # batches per core
T = 2048
D = 1024
U = 1024
CHUNK = 512  # t rows per chunk
N_CHUNKS = T // CHUNK  # 4
O_PER_CHUNK = CHUNK // 128  # 4 t-subtiles per chunk
NJ = D // 128  # 8 d-tiles
NM = U // 128  # 8 u-tiles

F32 = mybir.dt.float32
BF16 = mybir.dt.bfloat16
AX = mybir.AxisListType
AF = mybir.ActivationFunctionType


def build_kernel():
    nc = bacc.Bacc("TRN2", target_bir_lowering=False, debug=False,
                   num_devices=N_CORES)

    feats = nc.dram_tensor("features", [B_LOC, T, D], BF16, kind="ExternalInput")
    hidden = nc.dram_tensor("hidden", [B_LOC, D], BF16, kind="ExternalInput")
    w1 = nc.dram_tensor("W1", [D, U], BF16, kind="ExternalInput")
    w2 = nc.dram_tensor("W2", [D, U], BF16, kind="ExternalInput")
    b1 = nc.dram_tensor("b1", [U], F32, kind="ExternalInput")
    b2 = nc.dram_tensor("b2", [U], F32, kind="ExternalInput")
    v = nc.dram_tensor("V", [U, 1], F32, kind="ExternalInput")

    ctx_out = nc.dram_tensor("ctx", [B_LOC, D], F32, kind="ExternalOutput")
    w_out = nc.dram_tensor("w", [B_LOC, T], F32, kind="ExternalOutput")

    with tile.TileContext(nc) as tc:
        with (
            tc.tile_pool(name="const", bufs=1) as cpool,
            tc.tile_pool(name="fb", bufs=6) as fb_pool,
            tc.tile_pool(name="ft", bufs=5) as ft_pool,
            tc.tile_pool(name="score", bufs=3) as sc_pool,
            tc.tile_pool(name="small", bufs=2) as sm_pool,
            tc.tile_pool(name="acc", bufs=2) as acc_pool,
            tc.tile_pool(name="tmp", bufs=3) as tmp_pool,
            tc.tile_pool(name="ps_proj", bufs=5, space="PSUM") as ps_proj,
            tc.tile_pool(name="ps_lg", bufs=2, space="PSUM") as ps_lg,
            tc.tile_pool(name="ps_pst", bufs=1, space="PSUM") as ps_pst,
        ):
            # ---- constants / weights in SBUF --------------------------------
            ident1 = cpool.tile([1, 1], F32, tag="ident1")
            nc.vector.memset(ident1[:], 1.0)
            ones_col = cpool.tile([128, 1], BF16, tag="ones_col")
            nc.vector.memset(ones_col[:], 1.0)

            # the first two chunk transposes go first: the xbar-mode rule
            # serializes a transpose behind all in-flight DMA copies, so any
            # copy emitted before them would delay the first matmuls.
            pre_ft = {}
            for c0 in range(1):
                ftp = ft_pool.tile([128, NJ, CHUNK], BF16, tag="ft",
                                   name=f"ft_pre{c0}")
                nc.sync.dma_start_transpose(
                    ftp[:], feats[0, c0 * CHUNK:(c0 + 1) * CHUNK, :])
                pre_ft[(0, c0)] = ftp

            # small natural (contiguous) loads: they are cheap and feed
            # the bh build; scatter layouts are produced on-chip instead.
            hTn = cpool.tile([128, B_LOC, NJ], BF16, tag="ht")  # [dp, b, j]
            nc.gpsimd.dma_start(hTn[:], hidden.rearrange("b (j p) -> p b j", p=128))
            b1row = cpool.tile([1, U], F32, tag="b1row")
            nc.sync.dma_start(b1row[:], b1.rearrange("(o u) -> o u", o=1))
            b2row = cpool.tile([1, U], F32, tag="b2row")
            nc.sync.dma_start(b2row[:], b2.rearrange("(o u) -> o u", o=1))
            vrow = cpool.tile([1, U], F32, tag="vrow")
            nc.sync.dma_start(vrow[:], v.rearrange("u o -> o u"))

            w1sb = cpool.tile([128, NJ, U], BF16, tag="w1")  # [dp, j, u]
            for qf in range(4):
                nc.scalar.dma_start(
                    w1sb[:, qf * 2:(qf + 1) * 2, :],
                    w1[qf * 256:(qf + 1) * 256, :]
                    .rearrange("(j p) u -> p j u", p=128))

            # W2 slabs early so the bh matmuls don't stall mid-stream
            w2slabs = []
            for half in range(2):
                w2slab = ft_pool.tile([128, NJ, U // 2], BF16, tag="ft",
                                      name=f"w2slab{half}")
                nc.gpsimd.dma_start(
                    w2slab[:],
                    w2[:, half * 512:(half + 1) * 512]
                    .rearrange("(j p) u -> p j u", p=128),
                )
                w2slabs.append(w2slab)

            bh = cpool.tile([128, NM, B_LOC], F32, tag="bh")
            vsb = cpool.tile([128, NM], BF16, tag="v")  # [up, m]

            def build_bh():
                """bh[u, m, b] = (hidden @ W2)^T + b1 + b2; V to [up, m]."""
                # b1+b2 and V rows -> partition layout via K=1 PE transposes
                b12row = cpool.tile([1, U], F32, tag="b12row")
                nc.vector.tensor_add(b12row[:], b1row[:], b2row[:])
                psi = ps_proj.tile([128, CHUNK], F32, tag="proj", name="psinit")
                for m in range(NM):
                    nc.tensor.transpose(psi[:, m:m + 1],
                                        b12row[0:1, m * 128:(m + 1) * 128], ident1)
                for m in range(NM):
                    nc.tensor.transpose(psi[:, NM + m:NM + m + 1],
                                        vrow[0:1, m * 128:(m + 1) * 128], ident1)
                b12T = cpool.tile([128, NM], F32, tag="b12")
                nc.scalar.copy(b12T[:], psi[:, :NM])
                nc.scalar.copy(vsb[:], psi[:, NM:2 * NM])

                for half in range(2):
                    w2slab = w2slabs[half]
                    for mm in range(NM // 2):
                        m = half * 4 + mm
                        ps = ps_proj.tile([128, CHUNK], F32, tag="proj")
                        for j in range(NJ):
                            nc.tensor.matmul(
                                ps[:, :B_LOC],
                                lhsT=w2slab[:, j, mm * 128:(mm + 1) * 128],
                                rhs=hTn[:, :, j],
                                start=(j == 0),
                                stop=(j == NJ - 1),
                            )
                        nc.scalar.activation(bh[:, m, :], ps[:, :B_LOC],
                                             AF.Identity, bias=b12T[:, m:m + 1])

            # ---- main loop --------------------------------------------------
            def main_phase_chunk(b, c, la, acc, mid_hook=None):
                """Load + transpose + proj + tanh + logits for chunk c; defer
                the softmax tail + context matmuls to the next chunk."""
                # FT straight from DRAM through the xbar: ft[p, j, t] = F^T[j*128+p, t]
                ft = pre_ft.pop((b, c), None)
                if ft is None:
                    ft = ft_pool.tile([128, NJ, CHUNK], BF16, tag="ft")
                    nc.sync.dma_start_transpose(
                        ft[:], feats[b, c * CHUNK:(c + 1) * CHUNK, :])

                # natural-layout copy for the context matmuls (1-chunk slack)
                fb = fb_pool.tile([128, O_PER_CHUNK, D], BF16, tag="fb")
                nc.gpsimd.dma_start(
                    fb[:],
                    feats[b, c * CHUNK:(c + 1) * CHUNK, :]
                    .rearrange("(o p) d -> p o d", p=128),
                )

                # projT (per u-tile) -> tanh -> scoreT
                score = sc_pool.tile([128, NM, CHUNK], BF16, tag="score")
                for m in range(NM):
                    ps = ps_proj.tile([128, CHUNK], F32, tag="proj")
                    for j in range(NJ):
                        nc.tensor.matmul(
                            ps[:],
                            lhsT=w1sb[:, j, m * 128:(m + 1) * 128],
                            rhs=ft[:, j, :],
                            start=(j == 0),
                            stop=(j == NJ - 1),
                        )
                    if m == 0 and mid_hook is not None:
                        mid_hook()
                    nc.scalar.activation(score[:, m, :], ps[:], AF.Tanh,
                                         bias=bh[:, m, b:b + 1])

                psl = ps_lg.tile([1, CHUNK], F32, tag="lg")
                for m in range(NM):
                    nc.tensor.matmul(
                        psl[:],
                        lhsT=vsb[:, m:m + 1],
                        rhs=score[:, m, :],
                        start=(m == 0),
                        stop=(m == NM - 1),
                    )
                nc.scalar.copy(la[:, c * CHUNK:(c + 1) * CHUNK], psl[:])

                def post():
                    # logits chunk -> t-partition layout -> exp -> weighted
                    # accumulation of F rows into acc (context, pre-reduction)
                    pst = ps_pst.tile([128, O_PER_CHUNK], F32, tag="pst")
                    for o in range(O_PER_CHUNK):
                        blk = la[0:1, c * CHUNK + o * 128: c * CHUNK + (o + 1) * 128]
                        nc.tensor.transpose(pst[:, o:o + 1], blk, ident1)
                    ewc = sm_pool.tile([128, O_PER_CHUNK], F32, tag="ewc")
                    nc.scalar.activation(ewc[:], pst[:], AF.Exp)
                    for o in range(O_PER_CHUNK):
                        tmp = tmp_pool.tile([128, D], BF16, tag="tmp")
                        nc.scalar.activation(tmp[:], fb[:, o, :], AF.Copy,
                                             scale=ewc[:, o:o + 1])
                        if c == 0 and o == 0:
                            nc.vector.tensor_copy(acc[:], tmp[:])
                        else:
                            nc.vector.tensor_add(acc[:], acc[:], tmp[:])

                return post

            def batch_finish(b, la, acc):
                """exp + denominator, normalized weights out, scaled context out."""
                ew = sm_pool.tile([1, T], F32, tag="ew")
                ssum = sm_pool.tile([1, 1], F32, tag="ssum")
                nc.scalar.activation(ew[:], la[:], AF.Exp, accum_out=ssum[:])
                s_inv = sm_pool.tile([1, 1], F32, tag="sinv")
                nc.vector.reciprocal(s_inv[:], ssum[:])
                nc.vector.tensor_scalar_mul(ew[:], ew[:], s_inv[:, 0:1])
                nc.gpsimd.dma_start(w_out[b:b + 1, :], ew[:])

                ctx_sb = sm_pool.tile([1, D], F32, tag="ctx")
                for h in range(2):
                    psc = ps_lg.tile([1, CHUNK], F32, tag="lg")
                    nc.tensor.matmul(psc[:], lhsT=ones_col[:],
                                     rhs=acc[:, h * 512:(h + 1) * 512],
                                     start=True, stop=True)
                    nc.vector.tensor_scalar_mul(
                        ctx_sb[:, h * 512:(h + 1) * 512], psc[:],
                        s_inv[:, 0:1])
                nc.gpsimd.dma_start(ctx_out[b:b + 1, :], ctx_sb[:])

            pending = None  # previous chunk's deferred tail (one-chunk delay)
            for b in range(B_LOC):
                la = sm_pool.tile([1, T], F32, tag="la")
                acc = acc_pool.tile([128, D], BF16, tag="acc")
                for c in range(N_CHUNKS):
                    hook = build_bh if (b == 0 and c == 0) else None
                    tail = main_phase_chunk(b, c, la, acc, mid_hook=hook)
                    if pending is not None:
                        pending()
                    pending = tail
                # batch end: flush last chunk's tail, then finish
                pending()
                pending = None
                batch_finish(b, la, acc)

    nc.compile()
    return nc


_NC_CACHE = None


def _get_nc():
    global _NC_CACHE
    if _NC_CACHE is None:
        _NC_CACHE = build_kernel()
    return _NC_CACHE


def kernel(**inputs):
    bf16 = ml_dtypes.bfloat16
    feats = np.ascontiguousarray(np.asarray(inputs["features"]).astype(bf16))
    hidden = np.ascontiguousarray(np.asarray(inputs["hidden"]).astype(bf16))
    w1 = np.ascontiguousarray(np.asarray(inputs["W1"]).astype(bf16))
    w2 = np.ascontiguousarray(np.asarray(inputs["W2"]).astype(bf16))
    b1 = np.ascontiguousarray(np.asarray(inputs["b1"], dtype=np.float32))
    b2 = np.ascontiguousarray(np.asarray(inputs["b2"], dtype=np.float32))
    v = np.ascontiguousarray(np.asarray(inputs["V"], dtype=np.float32))

    nc = _get_nc()
    in_maps = []
    for i in range(N_CORES):
        sl = slice(i * B_LOC, (i + 1) * B_LOC)
        in_maps.append({
            "features": feats[sl],
            "hidden": hidden[sl],
            "W1": w1,
            "W2": w2,
            "b1": b1,
            "b2": b2,
            "V": v,
        })
    res = run_bass_kernel_spmd(nc, in_maps, core_ids=list(range(N_CORES)))

    ctx = np.concatenate([res.results[i]["ctx"] for i in range(N_CORES)], axis=0)
    w = np.concatenate([res.results[i]["w"] for i in range(N_CORES)], axis=0)
    return ctx, w.reshape(N_CORES * B_LOC, T, 1)


# revision 27
# speedup vs baseline: 1.0698x; 1.0561x over previous
"""Bahdanau attention kernel for Trainium2, SPMD over 8 NeuronCores.

Problem shapes: features [32, 2048, 1024] f32, hidden [32, 1024] f32,
W1/W2 [1024, 1024], b1/b2 [1024], V [1024, 1], bv [1].

Returns (context_vector [32, 1024] f32, attention_weights [32, 2048, 1] f32).

Sharding: data-parallel over batch B; each of the 8 cores handles 4 batches
end-to-end (no collectives needed).

Per-core pipeline, per batch b (T=2048 split into 4 chunks of 512 rows):
  1. SWDGE cast-DMA loads the F chunk f32->bf16 as FB [128(tp), 4(o), 1024(d)].
  2. DMA-xbar transposes the whole chunk: ft[p, o, j, c] = F^T[j*128+p, o*128+c].
  3. projT[u,t] = sum_j W1[dj,u].T @ FT[dj,t]  (bf16 matmul, PSUM f32).
  4. ScalarE tanh(projT + bh[u]) -> scoreT bf16, where bh = hidden@W2 + b1 + b2
     enters as the per-partition activation bias (free).
  5. logits[1,t] = sum_m V[um].T @ scoreT[um,t]  (matmul, M=1).
  6. Online softmax without max-subtraction (tanh bounds |logits| <= sum|V|
     ~ 25, so exp is safe in f32): per chunk, logits are PE-transposed to
     t-partition layout, exponentiated, and immediately contracted with FB
     into two running context PSUM accumulators [1, 512].  The chunk's
     transposes/exp/context matmuls are emitted one chunk later so the PE
     never waits on the ScalarE chain.
  7. Batch end: exp over the full logit row (accum_out gives the softmax
     denominator), reciprocal, normalize weights in place, scale the context
     accumulators.
"""

import ml_dtypes
import numpy as np

import concourse.bass as bass
import concourse.mybir as mybir
import concourse.tile as tile
from concourse import bacc
from concourse.bass_utils import run_bass_kernel_spmd

N_CORES = 8
B_LOC = 4  # batches per core
T = 2048
D = 1024
U = 1024
CHUNK = 512  # t rows per chunk
N_CHUNKS = T // CHUNK  # 4
O_PER_CHUNK = CHUNK // 128  # 4 t-subtiles per chunk
NJ = D // 128  # 8 d-tiles
NM = U // 128  # 8 u-tiles

F32 = mybir.dt.float32
BF16 = mybir.dt.bfloat16
AX = mybir.AxisListType
AF = mybir.ActivationFunctionType


def build_kernel():
    nc = bacc.Bacc("TRN2", target_bir_lowering=False, debug=False,
                   num_devices=N_CORES)

    feats = nc.dram_tensor("features", [B_LOC, T, D], BF16, kind="ExternalInput")
    hidden = nc.dram_tensor("hidden", [B_LOC, D], BF16, kind="ExternalInput")
    w1 = nc.dram_tensor("W1", [D, U], BF16, kind="ExternalInput")
    w2 = nc.dram_tensor("W2", [D, U], BF16, kind="ExternalInput")
    b1 = nc.dram_tensor("b1", [U], F32, kind="ExternalInput")
    b2 = nc.dram_tensor("b2", [U], F32, kind="ExternalInput")
    v = nc.dram_tensor("V", [U, 1], F32, kind="ExternalInput")

    ctx_out = nc.dram_tensor("ctx", [B_LOC, D], F32, kind="ExternalOutput")
    w_out = nc.dram_tensor("w", [B_LOC, T], F32, kind="ExternalOutput")

    with tile.TileContext(nc) as tc:
        with (
            tc.tile_pool(name="const", bufs=1) as cpool,
            tc.tile_pool(name="fb", bufs=6) as fb_pool,
            tc.tile_pool(name="ft", bufs=5) as ft_pool,
            tc.tile_pool(name="score", bufs=3) as sc_pool,
            tc.tile_pool(name="small", bufs=2) as sm_pool,
            tc.tile_pool(name="acc", bufs=2) as acc_pool,
            tc.tile_pool(name="tmp", bufs=3) as tmp_pool,
            tc.tile_pool(name="ps_proj", bufs=5, space="PSUM") as ps_proj,
            tc.tile_pool(name="ps_lg", bufs=2, space="PSUM") as ps_lg,
            tc.tile_pool(name="ps_pst", bufs=1, space="PSUM") as ps_pst,
        ):
            # ---- constants / weights in SBUF --------------------------------
            ident1 = cpool.tile([1, 1], F32, tag="ident1")
            nc.vector.memset(ident1[:], 1.0)
            ones_col = cpool.tile([128, 1], BF16, tag="ones_col")
            nc.vector.memset(ones_col[:], 1.0)

            # the first two chunk transposes go first: the xbar-mode rule
            # serializes a transpose behind all in-flight DMA copies, so any
            # copy emitted before them would delay the first matmuls.
            pre_ft = {}
            for c0 in range(1):
                ftp = ft_pool.tile([128, NJ, CHUNK], BF16, tag="ft",
                                   name=f"ft_pre{c0}")
                nc.sync.dma_start_transpose(
                    ftp[:], feats[0, c0 * CHUNK:(c0 + 1) * CHUNK, :])
                pre_ft[(0, c0)] = ftp

            # small natural (contiguous) loads: they are cheap and feed
            # the bh build; scatter layouts are produced on-chip instead.
            hTn = cpool.tile([128, B_LOC, NJ], BF16, tag="ht")  # [dp, b, j]
            nc.gpsimd.dma_start(hTn[:], hidden.rearrange("b (j p) -> p b j", p=128))
            b1row = cpool.tile([1, U], F32, tag="b1row")
            nc.sync.dma_start(b1row[:], b1.rearrange("(o u) -> o u", o=1))
            b2row = cpool.tile([1, U], F32, tag="b2row")
            nc.sync.dma_start(b2row[:], b2.rearrange("(o u) -> o u", o=1))
            vrow = cpool.tile([1, U], F32, tag="vrow")
            nc.sync.dma_start(vrow[:], v.rearrange("u o -> o u"))

            w1sb = cpool.tile([128, NJ, U], BF16, tag="w1")  # [dp, j, u]
            for qf in range(4):
                nc.scalar.dma_start(
                    w1sb[:, qf * 2:(qf + 1) * 2, :],
                    w1[qf * 256:(qf + 1) * 256, :]
                    .rearrange("(j p) u -> p j u", p=128))

            # W2 slabs early so the bh matmuls don't stall mid-stream
            w2slabs = []
            for half in range(2):
                w2slab = ft_pool.tile([128, NJ, U // 2], BF16, tag="ft",
                                      name=f"w2slab{half}")
                nc.gpsimd.dma_start(
                    w2slab[:],
                    w2[:, half * 512:(half + 1) * 512]
                    .rearrange("(j p) u -> p j u", p=128),
                )
                w2slabs.append(w2slab)

            bh = cpool.tile([128, NM, B_LOC], F32, tag="bh")
            vsb = cpool.tile([128, NM], BF16, tag="v")  # [up, m]

            def build_bh():
                """bh[u, m, b] = (hidden @ W2)^T + b1 + b2; V to [up, m]."""
                # b1+b2 and V rows -> partition layout via K=1 PE transposes
                b12row = cpool.tile([1, U], F32, tag="b12row")
                nc.vector.tensor_add(b12row[:], b1row[:], b2row[:])
                psi = ps_proj.tile([128, CHUNK], F32, tag="proj", name="psinit")
                for m in range(NM):
                    nc.tensor.transpose(psi[:, m:m + 1],
                                        b12row[0:1, m * 128:(m + 1) * 128], ident1)
                for m in range(NM):
                    nc.tensor.transpose(psi[:, NM + m:NM + m + 1],
                                        vrow[0:1, m * 128:(m + 1) * 128], ident1)
                b12T = cpool.tile([128, NM], F32, tag="b12")
                nc.scalar.copy(b12T[:], psi[:, :NM])
                nc.scalar.copy(vsb[:], psi[:, NM:2 * NM])

                for half in range(2):
                    w2slab = w2slabs[half]
                    for mm in range(NM // 2):
                        m = half * 4 + mm
                        ps = ps_proj.tile([128, CHUNK], F32, tag="proj")
                        for j in range(NJ):
                            nc.tensor.matmul(
                                ps[:, :B_LOC],
                                lhsT=w2slab[:, j, mm * 128:(mm + 1) * 128],
                                rhs=hTn[:, :, j],
                                start=(j == 0),
                                stop=(j == NJ - 1),
                            )
                        nc.scalar.activation(bh[:, m, :], ps[:, :B_LOC],
                                             AF.Identity, bias=b12T[:, m:m + 1])

            # ---- main loop --------------------------------------------------
            def main_phase_chunk(b, c, la, acc, mid_hook=None):
                """Load + transpose + proj + tanh + logits for chunk c; defer
                the softmax tail + context matmuls to the next chunk."""
                # FT straight from DRAM through the xbar: ft[p, j, t] = F^T[j*128+p, t]
                ft = pre_ft.pop((b, c), None)
                if ft is None:
                    ft = ft_pool.tile([128, NJ, CHUNK], BF16, tag="ft")
                    nc.sync.dma_start_transpose(
                        ft[:], feats[b, c * CHUNK:(c + 1) * CHUNK, :])

                # natural-layout copy for the context matmuls (1-chunk slack)
                fb = fb_pool.tile([128, O_PER_CHUNK, D], BF16, tag="fb")
                nc.gpsimd.dma_start(
                    fb[:],
                    feats[b, c * CHUNK:(c + 1) * CHUNK, :]
                    .rearrange("(o p) d -> p o d", p=128),
                )

                # projT (per u-tile) -> tanh -> scoreT
                score = sc_pool.tile([128, NM, CHUNK], BF16, tag="score")
                for m in range(NM):
                    ps = ps_proj.tile([128, CHUNK], F32, tag="proj")
                    for j in range(NJ):
                        nc.tensor.matmul(
                            ps[:],
                            lhsT=w1sb[:, j, m * 128:(m + 1) * 128],
                            rhs=ft[:, j, :],
                            start=(j == 0),
                            stop=(j == NJ - 1),
                        )
                    if m == 0 and mid_hook is not None:
                        mid_hook()
                    nc.scalar.activation(score[:, m, :], ps[:], AF.Tanh,
                                         bias=bh[:, m, b:b + 1])

                psl = ps_lg.tile([1, CHUNK], F32, tag="lg")
                for m in range(NM):
                    nc.tensor.matmul(
                        psl[:],
                        lhsT=vsb[:, m:m + 1],
                        rhs=score[:, m, :],
                        start=(m == 0),
                        stop=(m == NM - 1),
                    )
                nc.scalar.copy(la[:, c * CHUNK:(c + 1) * CHUNK], psl[:])

                def post():
                    # logits chunk -> t-partition layout -> exp -> weighted
                    # accumulation of F rows into acc (context, pre-reduction)
                    pst = ps_pst.tile([128, O_PER_CHUNK], F32, tag="pst")
                    for o in range(O_PER_CHUNK):
                        blk = la[0:1, c * CHUNK + o * 128: c * CHUNK + (o + 1) * 128]
                        nc.tensor.transpose(pst[:, o:o + 1], blk, ident1)
                    ewc = sm_pool.tile([128, O_PER_CHUNK], F32, tag="ewc")
                    nc.scalar.activation(ewc[:], pst[:], AF.Exp)
                    for o in range(O_PER_CHUNK):
                        tmp = tmp_pool.tile([128, D], BF16, tag="tmp")
                        nc.vector.tensor_scalar_mul(tmp[:], fb[:, o, :],
                                                    ewc[:, o:o + 1])
                        if c == 0 and o == 0:
                            nc.vector.tensor_copy(acc[:], tmp[:])
                        else:
                            nc.vector.tensor_add(acc[:], acc[:], tmp[:])

                return post

            def batch_finish(b, la, acc):
                """exp + denominator, normalized weights out, scaled context out."""
                ew = sm_pool.tile([1, T], F32, tag="ew")
                ssum = sm_pool.tile([1, 1], F32, tag="ssum")
                nc.scalar.activation(ew[:], la[:], AF.Exp, accum_out=ssum[:])
                s_inv = sm_pool.tile([1, 1], F32, tag="sinv")
                nc.vector.reciprocal(s_inv[:], ssum[:])
                nc.vector.tensor_scalar_mul(ew[:], ew[:], s_inv[:, 0:1])
                nc.gpsimd.dma_start(w_out[b:b + 1, :], ew[:])

                ctx_sb = sm_pool.tile([1, D], F32, tag="ctx")
                for h in range(2):
                    psc = ps_lg.tile([1, CHUNK], F32, tag="lg")
                    nc.tensor.matmul(psc[:], lhsT=ones_col[:],
                                     rhs=acc[:, h * 512:(h + 1) * 512],
                                     start=True, stop=True)
                    nc.vector.tensor_scalar_mul(
                        ctx_sb[:, h * 512:(h + 1) * 512], psc[:],
                        s_inv[:, 0:1])
                nc.gpsimd.dma_start(ctx_out[b:b + 1, :], ctx_sb[:])

            pending = None  # previous chunk's deferred tail (one-chunk delay)
            for b in range(B_LOC):
                la = sm_pool.tile([1, T], F32, tag="la")
                acc = acc_pool.tile([128, D], BF16, tag="acc")
                for c in range(N_CHUNKS):
                    hook = build_bh if (b == 0 and c == 0) else None
                    tail = main_phase_chunk(b, c, la, acc, mid_hook=hook)
                    if pending is not None:
                        pending()
                    pending = tail
                # batch end: flush last chunk's tail, then finish
                pending()
                pending = None
                batch_finish(b, la, acc)

    nc.compile()
    return nc


_NC_CACHE = None


def _get_nc():
    global _NC_CACHE
    if _NC_CACHE is None:
        _NC_CACHE = build_kernel()
    return _NC_CACHE


def kernel(**inputs):
    bf16 = ml_dtypes.bfloat16
    feats = np.ascontiguousarray(np.asarray(inputs["features"]).astype(bf16))
    hidden = np.ascontiguousarray(np.asarray(inputs["hidden"]).astype(bf16))
    w1 = np.ascontiguousarray(np.asarray(inputs["W1"]).astype(bf16))
    w2 = np.ascontiguousarray(np.asarray(inputs["W2"]).astype(bf16))
    b1 = np.ascontiguousarray(np.asarray(inputs["b1"], dtype=np.float32))
    b2 = np.ascontiguousarray(np.asarray(inputs["b2"], dtype=np.float32))
    v = np.ascontiguousarray(np.asarray(inputs["V"], dtype=np.float32))

    nc = _get_nc()
    in_maps = []
    for i in range(N_CORES):
        sl = slice(i * B_LOC, (i + 1) * B_LOC)
        in_maps.append({
            "features": feats[sl],
            "hidden": hidden[sl],
            "W1": w1,
            "W2": w2,
            "b1": b1,
            "b2": b2,
            "V": v,
        })
    res = run_bass_kernel_spmd(nc, in_maps, core_ids=list(range(N_CORES)))

    ctx = np.concatenate([res.results[i]["ctx"] for i in range(N_CORES)], axis=0)
    w = np.concatenate([res.results[i]["w"] for i in range(N_CORES)], axis=0)
    return ctx, w.reshape(N_CORES * B_LOC, T, 1)


# revision 29
# speedup vs baseline: 1.1571x; 1.0816x over previous
"""Bahdanau attention kernel for Trainium2, SPMD over 8 NeuronCores.

Problem shapes: features [32, 2048, 1024] f32, hidden [32, 1024] f32,
W1/W2 [1024, 1024], b1/b2 [1024], V [1024, 1], bv [1].

Returns (context_vector [32, 1024] f32, attention_weights [32, 2048, 1] f32).

Sharding: data-parallel over batch B; each of the 8 cores handles 4 batches
end-to-end (no collectives needed).

Per-core pipeline, per batch b (T=2048 split into 4 chunks of 512 rows):
  1. SWDGE cast-DMA loads the F chunk f32->bf16 as FB [128(tp), 4(o), 1024(d)].
  2. DMA-xbar transposes the whole chunk: ft[p, o, j, c] = F^T[j*128+p, o*128+c].
  3. projT[u,t] = sum_j W1[dj,u].T @ FT[dj,t]  (bf16 matmul, PSUM f32).
  4. ScalarE tanh(projT + bh[u]) -> scoreT bf16, where bh = hidden@W2 + b1 + b2
     enters as the per-partition activation bias (free).
  5. logits[1,t] = sum_m V[um].T @ scoreT[um,t]  (matmul, M=1).
  6. Online softmax without max-subtraction (tanh bounds |logits| <= sum|V|
     ~ 25, so exp is safe in f32): per chunk, logits are PE-transposed to
     t-partition layout, exponentiated, and immediately contracted with FB
     into two running context PSUM accumulators [1, 512].  The chunk's
     transposes/exp/context matmuls are emitted one chunk later so the PE
     never waits on the ScalarE chain.
  7. Batch end: exp over the full logit row (accum_out gives the softmax
     denominator), reciprocal, normalize weights in place, scale the context
     accumulators.
"""

import ml_dtypes
import numpy as np

import concourse.bass as bass
import concourse.mybir as mybir
import concourse.tile as tile
from concourse import bacc
from concourse.bass_utils import run_bass_kernel_spmd

N_CORES = 8
B_LOC = 4  # batches per core
T = 2048
D = 1024
U = 1024
CHUNK = 512  # t rows per chunk
N_CHUNKS = T // CHUNK  # 4
O_PER_CHUNK = CHUNK // 128  # 4 t-subtiles per chunk
NJ = D // 128  # 8 d-tiles
NM = U // 128  # 8 u-tiles

F32 = mybir.dt.float32
BF16 = mybir.dt.bfloat16
AX = mybir.AxisListType
AF = mybir.ActivationFunctionType


def build_kernel():
    nc = bacc.Bacc("TRN2", target_bir_lowering=False, debug=False,
                   num_devices=N_CORES)

    feats = nc.dram_tensor("features", [B_LOC, T, D], BF16, kind="ExternalInput")
    hidden = nc.dram_tensor("hidden", [B_LOC, D], BF16, kind="ExternalInput")
    w1 = nc.dram_tensor("W1", [D, U], BF16, kind="ExternalInput")
    w2 = nc.dram_tensor("W2", [D, U], BF16, kind="ExternalInput")
    b1 = nc.dram_tensor("b1", [U], F32, kind="ExternalInput")
    b2 = nc.dram_tensor("b2", [U], F32, kind="ExternalInput")
    v = nc.dram_tensor("V", [U, 1], F32, kind="ExternalInput")

    ctx_out = nc.dram_tensor("ctx", [B_LOC, D], F32, kind="ExternalOutput")
    w_out = nc.dram_tensor("w", [B_LOC, T], F32, kind="ExternalOutput")

    with tile.TileContext(nc) as tc:
        with (
            tc.tile_pool(name="const", bufs=1) as cpool,
            tc.tile_pool(name="fb", bufs=6) as fb_pool,
            tc.tile_pool(name="ft", bufs=5) as ft_pool,
            tc.tile_pool(name="score", bufs=3) as sc_pool,
            tc.tile_pool(name="small", bufs=2) as sm_pool,
            tc.tile_pool(name="acc", bufs=2) as acc_pool,
            tc.tile_pool(name="tmp", bufs=3) as tmp_pool,
            tc.tile_pool(name="ps_proj", bufs=5, space="PSUM") as ps_proj,
            tc.tile_pool(name="ps_lg", bufs=2, space="PSUM") as ps_lg,
            tc.tile_pool(name="ps_pst", bufs=1, space="PSUM") as ps_pst,
        ):
            # ---- constants / weights in SBUF --------------------------------
            ident1 = cpool.tile([1, 1], F32, tag="ident1")
            nc.vector.memset(ident1[:], 1.0)
            ones_col = cpool.tile([128, 1], BF16, tag="ones_col")
            nc.vector.memset(ones_col[:], 1.0)

            # the first two chunk transposes go first: the xbar-mode rule
            # serializes a transpose behind all in-flight DMA copies, so any
            # copy emitted before them would delay the first matmuls.
            pre_ft = {}
            for c0 in range(1):
                ftp = ft_pool.tile([128, NJ, CHUNK], BF16, tag="ft",
                                   name=f"ft_pre{c0}")
                nc.sync.dma_start_transpose(
                    ftp[:], feats[0, c0 * CHUNK:(c0 + 1) * CHUNK, :])
                pre_ft[(0, c0)] = ftp

            # small natural (contiguous) loads: they are cheap and feed
            # the bh build; scatter layouts are produced on-chip instead.
            hTn = cpool.tile([128, B_LOC, NJ], BF16, tag="ht")  # [dp, b, j]
            nc.gpsimd.dma_start(hTn[:], hidden.rearrange("b (j p) -> p b j", p=128))
            b1row = cpool.tile([1, U], F32, tag="b1row")
            nc.sync.dma_start(b1row[:], b1.rearrange("(o u) -> o u", o=1))
            b2row = cpool.tile([1, U], F32, tag="b2row")
            nc.sync.dma_start(b2row[:], b2.rearrange("(o u) -> o u", o=1))
            vrow = cpool.tile([1, U], F32, tag="vrow")
            nc.sync.dma_start(vrow[:], v.rearrange("u o -> o u"))

            w1sb = cpool.tile([128, NJ, U], BF16, tag="w1")  # [dp, j, u]
            for qf in range(4):
                nc.scalar.dma_start(
                    w1sb[:, qf * 2:(qf + 1) * 2, :],
                    w1[qf * 256:(qf + 1) * 256, :]
                    .rearrange("(j p) u -> p j u", p=128))

            # W2 slabs early so the bh matmuls don't stall mid-stream
            w2slabs = []
            for half in range(2):
                w2slab = ft_pool.tile([128, NJ, U // 2], BF16, tag="ft",
                                      name=f"w2slab{half}")
                nc.gpsimd.dma_start(
                    w2slab[:],
                    w2[:, half * 512:(half + 1) * 512]
                    .rearrange("(j p) u -> p j u", p=128),
                )
                w2slabs.append(w2slab)

            bh = cpool.tile([128, NM, B_LOC], F32, tag="bh")
            vsb = cpool.tile([128, NM], F32, tag="v")  # [up, m]

            def build_bh():
                """bh[u, m, b] = (hidden @ W2)^T + b1 + b2; V to [up, m]."""
                # b1+b2 and V rows -> partition layout via K=1 PE transposes
                b12row = cpool.tile([1, U], F32, tag="b12row")
                nc.vector.tensor_add(b12row[:], b1row[:], b2row[:])
                psi = ps_proj.tile([128, CHUNK], F32, tag="proj", name="psinit")
                for m in range(NM):
                    nc.tensor.transpose(psi[:, m:m + 1],
                                        b12row[0:1, m * 128:(m + 1) * 128], ident1)
                for m in range(NM):
                    nc.tensor.transpose(psi[:, NM + m:NM + m + 1],
                                        vrow[0:1, m * 128:(m + 1) * 128], ident1)
                b12T = cpool.tile([128, NM], F32, tag="b12")
                nc.scalar.copy(b12T[:], psi[:, :NM])
                nc.scalar.copy(vsb[:], psi[:, NM:2 * NM])

                for half in range(2):
                    w2slab = w2slabs[half]
                    for mm in range(NM // 2):
                        m = half * 4 + mm
                        ps = ps_proj.tile([128, CHUNK], F32, tag="proj")
                        for j in range(NJ):
                            nc.tensor.matmul(
                                ps[:, :B_LOC],
                                lhsT=w2slab[:, j, mm * 128:(mm + 1) * 128],
                                rhs=hTn[:, :, j],
                                start=(j == 0),
                                stop=(j == NJ - 1),
                            )
                        nc.scalar.activation(bh[:, m, :], ps[:, :B_LOC],
                                             AF.Identity, bias=b12T[:, m:m + 1])

            # ---- main loop --------------------------------------------------
            def main_phase_chunk(b, c, la, acc, mid_hook=None):
                """Load + transpose + proj + tanh + logits for chunk c; defer
                the softmax tail + context matmuls to the next chunk."""
                # FT straight from DRAM through the xbar: ft[p, j, t] = F^T[j*128+p, t]
                ft = pre_ft.pop((b, c), None)
                if ft is None:
                    ft = ft_pool.tile([128, NJ, CHUNK], BF16, tag="ft")
                    nc.sync.dma_start_transpose(
                        ft[:], feats[b, c * CHUNK:(c + 1) * CHUNK, :])

                # natural-layout copy for the context matmuls (1-chunk slack)
                fb = fb_pool.tile([128, O_PER_CHUNK, D], BF16, tag="fb")
                nc.gpsimd.dma_start(
                    fb[:],
                    feats[b, c * CHUNK:(c + 1) * CHUNK, :]
                    .rearrange("(o p) d -> p o d", p=128),
                )

                # projT (per u-tile) -> tanh -> scoreT; DVE scales by V[u]
                # and accumulates over u-tiles so the logits contraction is a
                # single ones-matmul instead of 8.
                score = sc_pool.tile([128, NM, CHUNK], BF16, tag="score")
                accl = sc_pool.tile([128, CHUNK], BF16, tag="accl")
                for m in range(NM):
                    ps = ps_proj.tile([128, CHUNK], F32, tag="proj")
                    for j in range(NJ):
                        nc.tensor.matmul(
                            ps[:],
                            lhsT=w1sb[:, j, m * 128:(m + 1) * 128],
                            rhs=ft[:, j, :],
                            start=(j == 0),
                            stop=(j == NJ - 1),
                        )
                    if m == 0 and mid_hook is not None:
                        mid_hook()
                    nc.scalar.activation(score[:, m, :], ps[:], AF.Tanh,
                                         bias=bh[:, m, b:b + 1])
                    if m == 0:
                        nc.vector.tensor_scalar_mul(accl[:], score[:, m, :],
                                                    vsb[:, m:m + 1])
                    else:
                        vtmp = tmp_pool.tile([128, CHUNK], BF16, tag="vtmp")
                        nc.vector.tensor_scalar_mul(vtmp[:], score[:, m, :],
                                                    vsb[:, m:m + 1])
                        nc.vector.tensor_add(accl[:], accl[:], vtmp[:])

                psl = ps_lg.tile([1, CHUNK], F32, tag="lg")
                nc.tensor.matmul(psl[:], lhsT=ones_col[:], rhs=accl[:],
                                 start=True, stop=True)
                nc.scalar.copy(la[:, c * CHUNK:(c + 1) * CHUNK], psl[:])

                def post():
                    # logits chunk -> t-partition layout -> exp -> weighted
                    # accumulation of F rows into acc (context, pre-reduction)
                    pst = ps_pst.tile([128, O_PER_CHUNK], F32, tag="pst")
                    for o in range(O_PER_CHUNK):
                        blk = la[0:1, c * CHUNK + o * 128: c * CHUNK + (o + 1) * 128]
                        nc.tensor.transpose(pst[:, o:o + 1], blk, ident1)
                    ewc = sm_pool.tile([128, O_PER_CHUNK], F32, tag="ewc")
                    nc.scalar.activation(ewc[:], pst[:], AF.Exp)
                    for o in range(O_PER_CHUNK):
                        tmp = tmp_pool.tile([128, D], BF16, tag="tmp")
                        nc.vector.tensor_scalar_mul(tmp[:], fb[:, o, :],
                                                    ewc[:, o:o + 1])
                        if c == 0 and o == 0:
                            nc.vector.tensor_copy(acc[:], tmp[:])
                        else:
                            nc.vector.tensor_add(acc[:], acc[:], tmp[:])

                return post

            def batch_finish(b, la, acc):
                """exp + denominator, normalized weights out, scaled context out."""
                ew = sm_pool.tile([1, T], F32, tag="ew")
                ssum = sm_pool.tile([1, 1], F32, tag="ssum")
                nc.scalar.activation(ew[:], la[:], AF.Exp, accum_out=ssum[:])
                s_inv = sm_pool.tile([1, 1], F32, tag="sinv")
                nc.vector.reciprocal(s_inv[:], ssum[:])
                nc.vector.tensor_scalar_mul(ew[:], ew[:], s_inv[:, 0:1])
                nc.gpsimd.dma_start(w_out[b:b + 1, :], ew[:])

                ctx_sb = sm_pool.tile([1, D], F32, tag="ctx")
                for h in range(2):
                    psc = ps_lg.tile([1, CHUNK], F32, tag="lg")
                    nc.tensor.matmul(psc[:], lhsT=ones_col[:],
                                     rhs=acc[:, h * 512:(h + 1) * 512],
                                     start=True, stop=True)
                    nc.vector.tensor_scalar_mul(
                        ctx_sb[:, h * 512:(h + 1) * 512], psc[:],
                        s_inv[:, 0:1])
                nc.gpsimd.dma_start(ctx_out[b:b + 1, :], ctx_sb[:])

            pending = None  # previous chunk's deferred tail (one-chunk delay)
            for b in range(B_LOC):
                la = sm_pool.tile([1, T], F32, tag="la")
                acc = acc_pool.tile([128, D], BF16, tag="acc")
                for c in range(N_CHUNKS):
                    hook = build_bh if (b == 0 and c == 0) else None
                    tail = main_phase_chunk(b, c, la, acc, mid_hook=hook)
                    if pending is not None:
                        pending()
                    pending = tail
                # batch end: flush last chunk's tail, then finish
                pending()
                pending = None
                batch_finish(b, la, acc)

    nc.compile()
    return nc


_NC_CACHE = None


def _get_nc():
    global _NC_CACHE
    if _NC_CACHE is None:
        _NC_CACHE = build_kernel()
    return _NC_CACHE


def kernel(**inputs):
    bf16 = ml_dtypes.bfloat16
    feats = np.ascontiguousarray(np.asarray(inputs["features"]).astype(bf16))
    hidden = np.ascontiguousarray(np.asarray(inputs["hidden"]).astype(bf16))
    w1 = np.ascontiguousarray(np.asarray(inputs["W1"]).astype(bf16))
    w2 = np.ascontiguousarray(np.asarray(inputs["W2"]).astype(bf16))
    b1 = np.ascontiguousarray(np.asarray(inputs["b1"], dtype=np.float32))
    b2 = np.ascontiguousarray(np.asarray(inputs["b2"], dtype=np.float32))
    v = np.ascontiguousarray(np.asarray(inputs["V"], dtype=np.float32))

    nc = _get_nc()
    in_maps = []
    for i in range(N_CORES):
        sl = slice(i * B_LOC, (i + 1) * B_LOC)
        in_maps.append({
            "features": feats[sl],
            "hidden": hidden[sl],
            "W1": w1,
            "W2": w2,
            "b1": b1,
            "b2": b2,
            "V": v,
        })
    res = run_bass_kernel_spmd(nc, in_maps, core_ids=list(range(N_CORES)))

    ctx = np.concatenate([res.results[i]["ctx"] for i in range(N_CORES)], axis=0)
    w = np.concatenate([res.results[i]["w"] for i in range(N_CORES)], axis=0)
    return ctx, w.reshape(N_CORES * B_LOC, T, 1)
